# revision 11
# baseline (speedup 1.0000x reference)
"""Per-sample batched matmul: out[b,o,f] = sum_i weights[b,o,i] * x[b,i,f].

Sharding: batch (bs=32) split across 8 NeuronCores, 4 samples each, zero
communication. Per sample the kernel PE-transposes W_b (fp32, exact) into
[I, O] layout, then runs accumulating float32r matmuls with x_b as the
moving operand (f32r = fp32 inputs truncated to ~fp22 in the PE, 4x the
fp32 matmul rate; accumulation stays fp32 in PSUM).
"""

import sys

sys.path.insert(0, "/opt/trn_rl_repo")

import numpy as np

BS, IN_SIZE, OUT_SIZE, FEATS = 32, 1024, 1024, 2048
N_CORES = 8
BPC = BS // N_CORES  # samples per core

P = 128
N_FREE = 512  # moving-operand free dim per matmul (1 PSUM bank of fp32)
KO = IN_SIZE // P  # 8 contraction tiles
MO = OUT_SIZE // P  # 8 output-row tiles
NF = FEATS // N_FREE  # 4 output-col chunks

_NC_CACHE = {}


def _build_nc(mm_dtype_name="float32r"):
    import concourse.mybir as mybir
    import concourse.tile as tile
    from concourse import bacc
    from concourse.masks import make_identity

    mm_dt = getattr(mybir.dt, mm_dtype_name)

    nc = bacc.Bacc("TRN2", target_bir_lowering=False, debug=False)
    x_d = nc.dram_tensor(
        "x", [BPC, IN_SIZE, FEATS], mybir.dt.float32, kind="ExternalInput"
    ).ap()
    w_d = nc.dram_tensor(
        "w", [BPC, OUT_SIZE, IN_SIZE], mybir.dt.float32, kind="ExternalInput"
    ).ap()
    o_d = nc.dram_tensor(
        "out", [BPC, OUT_SIZE, FEATS], mybir.dt.float32, kind="ExternalOutput"
    ).ap()

    with tile.TileContext(nc) as tc:
        with (
            tc.tile_pool(name="const", bufs=1) as const,
            tc.tile_pool(name="wn_pool", bufs=8) as wn_pool,
            tc.tile_pool(name="wt_pool", bufs=2) as wt_pool,
            tc.tile_pool(name="xn_pool", bufs=4) as xn_pool,
            tc.tile_pool(name="ot_pool", bufs=6) as ot_pool,
            tc.tile_pool(name="psmm", bufs=4, space="PSUM") as psmm_pool,
            tc.tile_pool(name="pstr", bufs=4, space="PSUM") as pstr_pool,
        ):
            ident32 = const.tile([P, P], mybir.dt.float32)
            make_identity(nc, ident32)
            ident = const.tile([P, P], mm_dt, name="identr")
            nc.vector.tensor_copy(out=ident[:], in_=ident32[:])

            TG = 4  # transposes packed per PSUM bank

            # Warm the PE (HAM un-throttle needs ~3.4us of sustained activity)
            # with dummy identity transposes while the first W/x DMAs land.
            # Consumed by tiny DVE copies so nothing dead-code-eliminates them.
            warm_sink = const.tile([P, 16], mm_dt, name="warm_sink")
            for wg in range(12):
                ptw = pstr_pool.tile([P, TG * P], mm_dt, tag="pt", name=f"ptw_{wg}")
                for c in range(TG):
                    nc.tensor.transpose(
                        ptw[:, c * P : (c + 1) * P], ident[:], ident[:]
                    )
                nc.vector.tensor_copy(out=warm_sink[:], in_=ptw[:, :16])

            def load_xn(b, n, x_r):
                """x chunk [128, KO, 512], split across 2 DMA queues."""
                xn = xn_pool.tile(
                    [P, KO, N_FREE], mm_dt, tag="xn", name=f"xn_{b}_{n}"
                )
                h = KO // 2
                src = x_r[:, :, n * N_FREE : (n + 1) * N_FREE].bitcast(mm_dt)
                nc.sync.dma_start(xn[:, :h], src[:, :h])
                nc.sync.dma_start(xn[:, h:], src[:, h:])
                return xn

            for b in range(BPC):
                x_r = x_d[b].rearrange("(ko p) f -> p ko f", p=P)
                xn_next = None
                if b > 0:
                    # prefetch this sample's first x chunk before the W burst
                    xn_next = load_xn(b, 0, x_r)

                # --- transpose W_b: [O, I] -> [I, O], tiled [128,128] ---
                # Single-pass f32r transposes; 4 results share one PSUM bank
                # and leave via one wide cast-copy, alternating DVE/ACT.
                wt = wt_pool.tile(
                    [P, KO, MO, P], mm_dt, tag="wt", name=f"wt_{b}"
                )
                for mo in range(MO):
                    wn = wn_pool.tile(
                        [P, IN_SIZE], mm_dt, tag="wn", name=f"wn_{b}_{mo}"
                    )
                    src = w_d[b, mo * P : (mo + 1) * P, :].bitcast(mm_dt)
                    nc.sync.dma_start(wn[:, : IN_SIZE // 2], src[:, : IN_SIZE // 2])
                    nc.sync.dma_start(wn[:, IN_SIZE // 2 :], src[:, IN_SIZE // 2 :])
                    if b == 0 and mo == 0:
                        # sample 0: W first (PE needs it first), x right behind
                        xn_next = load_xn(b, 0, x_r)
                    for g in range(KO // TG):
                        pt = pstr_pool.tile(
                            [P, TG * P],
                            mm_dt,
                            tag="pt",
                            name=f"pt_{b}_{mo}_{g}",
                        )
                        for c in range(TG):
                            ko = g * TG + c
                            nc.tensor.transpose(
                                pt[:, c * P : (c + 1) * P],
                                wn[:, ko * P : (ko + 1) * P],
                                ident[:],
                            )
                        dst = wt[:, g * TG : (g + 1) * TG, mo, :]
                        srcp = pt[:].rearrange("p (c q) -> p c q", c=TG)
                        if (mo * 2 + g) % 2 == 0:
                            nc.vector.tensor_copy(out=dst, in_=srcp)
                        else:
                            nc.scalar.copy(dst, srcp)

                # --- matmuls: out_b[mo, n] = sum_ko W_b.T[ko,mo].T @ x_b[ko,n] ---
                for n in range(NF):
                    xn = xn_next
                    if n + 1 < NF:
                        xn_next = load_xn(b, n + 1, x_r)
                    for mo in range(MO):
                        ps = psmm_pool.tile(
                            [P, N_FREE],
                            mybir.dt.float32,
                            tag="ps",
                            name=f"ps_{b}_{n}_{mo}",
                        )
                        for ko in range(KO):
                            nc.tensor.matmul(
                                ps[:],
                                wt[:, ko, mo, :],
                                xn[:, ko, :],
                                start=(ko == 0),
                                stop=(ko == KO - 1),
                            )
                        ot = ot_pool.tile(
                            [P, N_FREE],
                            mybir.dt.float32,
                            tag="ot",
                            name=f"ot_{b}_{n}_{mo}",
                        )
                        nc.vector.tensor_copy(out=ot[:], in_=ps[:])
                        # outputs ride GpSimd (SWDGE) and Scalar (2nd HWDGE
                        # ring), never Sync: their waits must not head-of-line
                        # block input prefetch
                        oeng = nc.gpsimd if mo % 2 == 0 else nc.scalar
                        oeng.dma_start(
                            o_d[b, mo * P : (mo + 1) * P, n * N_FREE : (n + 1) * N_FREE],
                            ot[:],
                        )

    nc.compile()
    return nc


def run(x, weights, trace=False):
    """Shard on batch, run SPMD on 8 cores, gather. Returns (out, results)."""
    from concourse.bass_utils import run_bass_kernel_spmd

    key = "nc"
    if key not in _NC_CACHE:
        _NC_CACHE[key] = _build_nc()
    nc = _NC_CACHE[key]

    x = np.ascontiguousarray(np.asarray(x, dtype=np.float32))
    weights = np.ascontiguousarray(np.asarray(weights, dtype=np.float32))
    in_maps = [
        {
            "x": x[c * BPC : (c + 1) * BPC],
            "w": weights[c * BPC : (c + 1) * BPC],
        }
        for c in range(N_CORES)
    ]
    res = run_bass_kernel_spmd(
        nc, in_maps, core_ids=list(range(N_CORES)), trace=trace
    )
    out = np.concatenate([res.results[c]["out"] for c in range(N_CORES)], axis=0)
    return out, res


def kernel(x, weights):
    out, _ = run(x, weights, trace=False)
    return out


# revision 13
# speedup vs baseline: 1.0444x; 1.0444x over previous
"""Per-sample batched matmul: out[b,o,f] = sum_i weights[b,o,i] * x[b,i,f].

Sharding: batch (bs=32) split across 8 NeuronCores, 4 samples each, zero
communication. Per sample the kernel PE-transposes W_b (fp32, exact) into
[I, O] layout, then runs accumulating float32r matmuls with x_b as the
moving operand (f32r = fp32 inputs truncated to ~fp22 in the PE, 4x the
fp32 matmul rate; accumulation stays fp32 in PSUM).
"""

import sys

sys.path.insert(0, "/opt/trn_rl_repo")

import numpy as np

BS, IN_SIZE, OUT_SIZE, FEATS = 32, 1024, 1024, 2048
N_CORES = 8
BPC = BS // N_CORES  # samples per core

P = 128
N_FREE = 512  # moving-operand free dim per matmul (1 PSUM bank of fp32)
KO = IN_SIZE // P  # 8 contraction tiles
MO = OUT_SIZE // P  # 8 output-row tiles
NF = FEATS // N_FREE  # 4 output-col chunks

_NC_CACHE = {}


def _build_nc(mm_dtype_name="float32r"):
    import concourse.mybir as mybir
    import concourse.tile as tile
    from concourse import bacc
    from concourse.masks import make_identity

    mm_dt = getattr(mybir.dt, mm_dtype_name)

    nc = bacc.Bacc("TRN2", target_bir_lowering=False, debug=False)
    x_d = nc.dram_tensor(
        "x", [BPC, IN_SIZE, FEATS], mybir.dt.float32, kind="ExternalInput"
    ).ap()
    w_d = nc.dram_tensor(
        "w", [BPC, OUT_SIZE, IN_SIZE], mybir.dt.float32, kind="ExternalInput"
    ).ap()
    o_d = nc.dram_tensor(
        "out", [BPC, OUT_SIZE, FEATS], mybir.dt.float32, kind="ExternalOutput"
    ).ap()

    with tile.TileContext(nc) as tc:
        with (
            tc.tile_pool(name="const", bufs=1) as const,
            tc.tile_pool(name="wn_pool", bufs=8) as wn_pool,
            tc.tile_pool(name="wt_pool", bufs=2) as wt_pool,
            tc.tile_pool(name="xn_pool", bufs=4) as xn_pool,
            tc.tile_pool(name="ot_pool", bufs=6) as ot_pool,
            tc.tile_pool(name="psmm", bufs=4, space="PSUM") as psmm_pool,
            tc.tile_pool(name="pstr", bufs=4, space="PSUM") as pstr_pool,
        ):
            ident32 = const.tile([P, P], mybir.dt.float32)
            make_identity(nc, ident32)
            ident = const.tile([P, P], mm_dt, name="identr")
            nc.vector.tensor_copy(out=ident[:], in_=ident32[:])

            TG = 4  # transposes packed per PSUM bank

            def load_xn(b, n, x_r):
                """x chunk [128, KO, 512], split across 2 DMA queues."""
                xn = xn_pool.tile(
                    [P, KO, N_FREE], mm_dt, tag="xn", name=f"xn_{b}_{n}"
                )
                h = KO // 2
                src = x_r[:, :, n * N_FREE : (n + 1) * N_FREE].bitcast(mm_dt)
                nc.sync.dma_start(xn[:, :h], src[:, :h])
                nc.sync.dma_start(xn[:, h:], src[:, h:])
                return xn

            def transpose_w_block(b, mo, wn, wt):
                """8 f32r transposes of one W row-block; 4 share a PSUM bank,
                leaving via one wide cast-copy, alternating DVE/ACT."""
                for g in range(KO // TG):
                    pt = pstr_pool.tile(
                        [P, TG * P], mm_dt, tag="pt", name=f"pt_{b}_{mo}_{g}"
                    )
                    for c in range(TG):
                        ko = g * TG + c
                        nc.tensor.transpose(
                            pt[:, c * P : (c + 1) * P],
                            wn[:, ko * P : (ko + 1) * P],
                            ident[:],
                        )
                    dst = wt[:, g * TG : (g + 1) * TG, mo, :]
                    srcp = pt[:].rearrange("p (c q) -> p c q", c=TG)
                    if (mo * 2 + g) % 2 == 0:
                        nc.vector.tensor_copy(out=dst, in_=srcp)
                    else:
                        nc.scalar.copy(dst, srcp)

            def load_wn(b, mo):
                wn = wn_pool.tile(
                    [P, IN_SIZE], mm_dt, tag="wn", name=f"wn_{b}_{mo}"
                )
                src = w_d[b, mo * P : (mo + 1) * P, :].bitcast(mm_dt)
                nc.sync.dma_start(wn[:, : IN_SIZE // 2], src[:, : IN_SIZE // 2])
                nc.sync.dma_start(wn[:, IN_SIZE // 2 :], src[:, IN_SIZE // 2 :])
                return wn

            def mm_group(b, n, mo, xn, wt):
                """One [128, 512] output tile: 8 accumulating matmuls,
                DVE psum eviction, output DMA on GpSimd (SWDGE) so its waits
                never head-of-line block input prefetch on Sync."""
                ps = psmm_pool.tile(
                    [P, N_FREE], mybir.dt.float32, tag="ps", name=f"ps_{b}_{n}_{mo}"
                )
                for ko in range(KO):
                    nc.tensor.matmul(
                        ps[:],
                        wt[:, ko, mo, :],
                        xn[:, ko, :],
                        start=(ko == 0),
                        stop=(ko == KO - 1),
                    )
                ot = ot_pool.tile(
                    [P, N_FREE], mybir.dt.float32, tag="ot", name=f"ot_{b}_{n}_{mo}"
                )
                nc.vector.tensor_copy(out=ot[:], in_=ps[:])
                nc.gpsimd.dma_start(
                    o_d[b, mo * P : (mo + 1) * P, n * N_FREE : (n + 1) * N_FREE],
                    ot[:],
                )

            for b in range(BPC):
                x_r = x_d[b].rearrange("(ko p) f -> p ko f", p=P)
                wt = wt_pool.tile(
                    [P, KO, MO, P], mm_dt, tag="wt", name=f"wt_{b}"
                )
                if b == 0:
                    # Startup is DMA-limited: interleave each W block's
                    # transposes with its first MM group so the PE paces with
                    # the arriving data instead of waiting for all of W.
                    xn_cur = None
                    for mo in range(MO):
                        wn = load_wn(b, mo)
                        if mo == 0:
                            xn_cur = load_xn(b, 0, x_r)
                        transpose_w_block(b, mo, wn, wt)
                        mm_group(b, 0, mo, xn_cur, wt)
                        if mo == 4:
                            xn_next = load_xn(b, 1, x_r)
                    start_n = 1
                else:
                    # steady state: x chunk prefetched ahead of the W burst
                    xn_next = load_xn(b, 0, x_r)
                    for mo in range(MO):
                        wn = load_wn(b, mo)
                        transpose_w_block(b, mo, wn, wt)
                    start_n = 0

                for n in range(start_n, NF):
                    xn = xn_next
                    if n + 1 < NF:
                        xn_next = load_xn(b, n + 1, x_r)
                    for mo in range(MO):
                        mm_group(b, n, mo, xn, wt)

    nc.compile()
    return nc


def run(x, weights, trace=False):
    """Shard on batch, run SPMD on 8 cores, gather. Returns (out, results)."""
    from concourse.bass_utils import run_bass_kernel_spmd

    key = "nc"
    if key not in _NC_CACHE:
        _NC_CACHE[key] = _build_nc()
    nc = _NC_CACHE[key]

    x = np.ascontiguousarray(np.asarray(x, dtype=np.float32))
    weights = np.ascontiguousarray(np.asarray(weights, dtype=np.float32))
    in_maps = [
        {
            "x": x[c * BPC : (c + 1) * BPC],
            "w": weights[c * BPC : (c + 1) * BPC],
        }
        for c in range(N_CORES)
    ]
    res = run_bass_kernel_spmd(
        nc, in_maps, core_ids=list(range(N_CORES)), trace=trace
    )
    out = np.concatenate([res.results[c]["out"] for c in range(N_CORES)], axis=0)
    return out, res


def kernel(x, weights):
    out, _ = run(x, weights, trace=False)
    return out
